# revision 1
# baseline (speedup 1.0000x reference)
"""Trainium2 Bass kernel for the CAM (channel attention) module.

Computes, per batch element b:
    q = x[b].reshape(C, N)                      # C=512, N=4096
    E = q @ q.T                                 # C x C  (symmetric)
    att = softmax(rowmax(E) - E, axis=-1)       # == softmax(-E) row-wise
    out = gamma * (att @ q) + x[b]

Sharding: data-parallel over batch. 16 batch elements -> 2 per NeuronCore
across 8 cores. gamma replicated. No collectives.

Per-core kernel strategy (per batch element):
  1. DMA q into SBUF in natural layout qnat[c_part, n_free] (fp32, exact bits
     are reused for the +x residual, so this tile is never rounded).
  2. Build qT[n_part, c_free] with 128 PE [128x128] transposes; 4 transposes
     share one PSUM bank so a single [128,512] DVE copy drains them (4x fewer
     DVE ops). qT is stored as float32r: the DVE copy rounds, satisfying the
     fp32r-producer rule, and the energy matmul then runs at full PE rate
     (1 cycle/row) instead of fp32's 1/4 rate.
  3. E tiles [128, 512] accumulate in PSUM via fp32r matmuls.
  4. Column-oriented softmax avoids transposing the attention matrix:
     att_T[d, c] = exp(min_c - E[d, c]) / R_c with R_c = sum_d exp(...).
     E is symmetric so min_c (row mins) equals the column-min vector; the
     stored E tile read with d on partitions is already att_T-oriented.
     exp argument <= 0 always, so no overflow; R is clamped before the
     reciprocal so no NaN is possible.
  5. U = exp(min_c - E) in bf16 is the stationary operand of the value
     matmul against a bf16 copy of q (cast on the idle scalar engine).
     gamma/R_c (per output partition) and the +x residual are fused into one
     DVE scalar_tensor_tensor per output chunk; x enters only here, in exact
     fp32, so for gamma == 0 the kernel output is bit-exact x.
"""

import sys

import numpy as np

_REPO = "/opt/trn_rl_repo"
if _REPO not in sys.path:
    sys.path.insert(0, _REPO)

B_TOTAL, C, H, W = 16, 512, 64, 64
N = H * W          # 4096
NCORES = 8
B = B_TOTAL // NCORES  # batches per core = 2
CT = C // 128      # 4 c-tiles
NT = N // 128      # 32 n-tiles
NCH = N // 512     # 8 output column chunks

_cache = {}


def _build_program():
    import concourse.bass as bass
    import concourse.bacc as bacc
    import concourse.mybir as mybir
    import concourse.tile as tile
    from contextlib import ExitStack

    f32 = mybir.dt.float32
    f32r = mybir.dt.float32r
    bf16 = mybir.dt.bfloat16
    AX = mybir.AxisListType
    OP = mybir.AluOpType
    ACT = mybir.ActivationFunctionType

    nc = bacc.Bacc("TRN2", target_bir_lowering=False, debug=False)

    x = nc.dram_tensor("x", [B, C, N], f32, kind="ExternalInput").ap()
    g128 = nc.dram_tensor("gamma128", [128, 1], f32, kind="ExternalInput").ap()
    ident_d = nc.dram_tensor("ident", [128, 128], f32, kind="ExternalInput").ap()
    y = nc.dram_tensor("y", [B, C, N], f32, kind="ExternalOutput").ap()

    with ExitStack() as ctx:
        tc = ctx.enter_context(tile.TileContext(nc))
        const_p = ctx.enter_context(tc.tile_pool(name="const", bufs=1))
        # qnat (fp32 q) and qT (f32r transposed q) alternate through 2 slots;
        # batch b+1's qnat lands in the slot freed by batch b's qT so its DMA
        # overlaps batch b's value-matmul phase.
        big_p = ctx.enter_context(tc.tile_pool(name="big", bufs=2))
        q_p = ctx.enter_context(tc.tile_pool(name="qq", bufs=1))
        qbf_p = ctx.enter_context(tc.tile_pool(name="qbf", bufs=1))
        tmp_p = ctx.enter_context(tc.tile_pool(name="tmp", bufs=2))
        sm_p = ctx.enter_context(tc.tile_pool(name="sm", bufs=2))
        rep_p = ctx.enter_context(tc.tile_pool(name="rep", bufs=1))
        osb_p = ctx.enter_context(tc.tile_pool(name="osb", bufs=8))
        ps = ctx.enter_context(tc.tile_pool(name="ps", bufs=8, space="PSUM"))

        ident = const_p.tile([128, 128], f32, tag="ident")
        nc.sync.dma_start(ident[:], ident_d)
        gam = const_p.tile([128, 1], f32, tag="gam")
        nc.sync.dma_start(gam[:], g128)
        ones128 = const_p.tile([128, 1], bf16, tag="ones128")
        nc.gpsimd.memset(ones128[:], 1.0)
        ones1 = const_p.tile([1, 128], f32, tag="ones1")
        nc.gpsimd.memset(ones1[:], 1.0)

        # warm the PE clock during the initial DMA wait: dummy transposes of
        # the identity keep the ramp/HAM window busy so the first real
        # transposes run at full clock
        warm = ps.tile([128, 512], f32, tag="ps", name="warm")
        for w in range(8):
            nc.tensor.matmul(
                warm[:, 128 * (w % 4):128 * (w % 4 + 1)],
                ident[:],
                ident[:],
                is_transpose=True,
                skip_group_check=True,
            )

        for b in range(B):
            # ---- load q in natural layout, chunked so transposes can
            #      start as soon as the first columns land
            qnat = big_p.tile([128, CT, N], f32, tag="big")
            for t in range(CT):
                for lo, hi in [(0, 128), (128, 512)]:
                    nc.sync.dma_start(
                        qnat[:, t, lo:hi],
                        x[b, 128 * t:128 * (t + 1), lo:hi],
                    )
                for h in range(1, 8):
                    nc.sync.dma_start(
                        qnat[:, t, 512 * h:512 * (h + 1)],
                        x[b, 128 * t:128 * (t + 1), 512 * h:512 * (h + 1)],
                    )

            # ---- build qT[n_part, c_free]; 4 transposes per PSUM bank, one
            #      [128,512] DVE copy per bank (rounds to f32r)
            qt = big_p.tile([128, NT, C], f32r, tag="big")
            for t in range(CT):
                for jq in range(NT // 4):
                    tp4 = ps.tile([128, 512], f32, tag="ps")
                    for i in range(4):
                        j = 4 * jq + i
                        nc.tensor.matmul(
                            tp4[:, 128 * i:128 * (i + 1)],
                            qnat[:, t, 128 * j:128 * (j + 1)],
                            ident[:],
                            is_transpose=True,
                            skip_group_check=True,
                        )
                    nc.vector.tensor_copy(
                        qt[:, 4 * jq:4 * (jq + 1), 128 * t:128 * (t + 1)],
                        tp4[:].rearrange("p (a c) -> p a c", a=4),
                    )

            # ---- bf16 copy of q for the value matmul, on the idle scalar
            #      engine (ACT)
            qbf = qbf_p.tile([128, CT, N], bf16, tag="qbf")
            for t in range(CT):
                nc.scalar.copy(qbf[:, t, :], qnat[:, t, :])

            # ---- energy: E is symmetric, so compute only columns
            #      [lo_t:512] per row-tile (lo capped at 256: narrower f32r
            #      moving operands drop to 1/4 rate) and mirror the missing
            #      [128,128] blocks by transposing the stored ones.
            elo = [0, 128, 256, 256]
            mirrors = {0: [(0, 1), (0, 2), (0, 3)], 1: [(1, 2), (1, 3)]}
            rmins = sm_p.tile([128, CT], f32, tag="rmins")
            colrep_ps = ps.tile([128, C], f32, tag="ps")
            E = [ps.tile([128, C], f32, tag="ps", name=f"Et{t_}")
                 for t_ in range(CT)]
            for t in range(CT):
                Et = E[t]
                for j in range(NT):
                    nc.tensor.matmul(
                        Et[:, elo[t]:C],
                        qt[:, j, 128 * t:128 * (t + 1)],
                        qt[:, j, elo[t]:C],
                        start=(j == 0),
                        stop=(j == NT - 1),
                    )
                # mirror blocks sourced from tile t into later tiles' banks
                # (target regions are disjoint from their MM-written ranges,
                # so this can precede those tiles' accumulation)
                for s, tt in mirrors.get(t, []):
                    blk = sm_p.tile([128, 128], f32, tag="mirror")
                    nc.vector.tensor_copy(
                        blk[:], E[s][:, 128 * tt:128 * (tt + 1)]
                    )
                    nc.tensor.matmul(
                        E[tt][:, 128 * s:128 * (s + 1)],
                        blk[:],
                        ident[:],
                        is_transpose=True,
                        skip_group_check=True,
                    )
                # tile t of E is now complete (its own MMs + any mirrors
                # emitted in earlier iterations): fold its stats immediately
                # so only tile 3's chain trails the energy phase
                nc.vector.tensor_reduce(
                    rmins[:, t:t + 1], E[t][:], axis=AX.X, op=OP.min
                )
                tpm = ps.tile([1, 128], f32, tag="ps")
                nc.tensor.transpose(tpm[:], rmins[:, t:t + 1], ident[:])
                stT = sm_p.tile([1, 128], f32, tag="stT")
                nc.vector.tensor_copy(stT[:], tpm[:])
                nc.tensor.matmul(
                    colrep_ps[:, 128 * t:128 * (t + 1)],
                    ones1[:],
                    stT[:],
                    start=True,
                    stop=True,
                )
            colrep = rep_p.tile([128, C], f32, tag="colrep")
            nc.vector.tensor_copy(colrep[:], colrep_ps[:])

            # ---- U[d, c] = exp(min_c - E[d, c])  (<= 1, no overflow)
            U = q_p.tile([128, CT, C], bf16, tag="qq")
            for t in range(CT):
                tmp = tmp_p.tile([128, C], f32, tag="tmp")
                nc.vector.tensor_tensor(
                    tmp[:], colrep[:], E[t][:], op=OP.subtract
                )
                nc.scalar.activation(U[:, t, :], tmp[:], ACT.Exp)

            # ---- out[c, n] = scale_c * sum_d U[d, c] q[d, n] + x[c, n]
            #      R_c = sum_d U[d, c] (PE ones-reduction) is interleaved
            #      per m so the first value matmuls start sooner;
            #      scale_m = gamma / max(R, tiny) per output partition
            for m in range(CT):
                Rp = ps.tile([128, 1], f32, tag="ps")
                for k in range(CT):
                    nc.tensor.matmul(
                        Rp[:],
                        U[:, k, 128 * m:128 * (m + 1)],
                        ones128[:],
                        start=(k == 0),
                        stop=(k == CT - 1),
                    )
                Rsb = sm_p.tile([128, 1], f32, tag="rsb")
                nc.vector.tensor_scalar_max(Rsb[:], Rp[:], 1e-38)
                rec = sm_p.tile([128, 1], f32, tag="rec")
                nc.vector.reciprocal(rec[:], Rsb[:])
                sc = sm_p.tile([128, 1], f32, tag=f"scale{m}")
                nc.vector.tensor_scalar_mul(sc[:], rec[:], gam[:, 0:1])
                O = []
                for n in range(NCH):
                    On = ps.tile([128, 512], f32, tag="ps")
                    O.append(On)
                for k in range(CT):
                    for n in range(NCH):
                        nc.tensor.matmul(
                            O[n][:],
                            U[:, k, 128 * m:128 * (m + 1)],
                            qbf[:, k, 512 * n:512 * (n + 1)],
                            start=(k == 0),
                            stop=(k == CT - 1),
                            skip_group_check=True,
                        )
                for n in range(NCH):
                    osb = osb_p.tile([128, 512], f32, tag="osb")
                    nc.vector.scalar_tensor_tensor(
                        osb[:],
                        O[n][:],
                        sc[:],
                        qnat[:, m, 512 * n:512 * (n + 1)],
                        op0=OP.mult,
                        op1=OP.add,
                    )
                    nc.sync.dma_start(
                        y[b, 128 * m:128 * (m + 1), 512 * n:512 * (n + 1)],
                        osb[:],
                    )

    nc.compile()
    return nc


def get_program():
    if "nc" not in _cache:
        _cache["nc"] = _build_program()
    return _cache["nc"]


def kernel(x, gamma):
    from concourse.bass_utils import run_bass_kernel_spmd

    nc = get_program()
    xr = np.ascontiguousarray(
        np.asarray(x, dtype=np.float32).reshape(B_TOTAL, C, N)
    )
    g = np.asarray(gamma, dtype=np.float32).reshape(1)
    g128 = np.ascontiguousarray(
        np.broadcast_to(g.reshape(1, 1), (128, 1))
    ).astype(np.float32)
    ident = np.eye(128, dtype=np.float32)
    in_maps = [
        {
            "x": xr[i * B:(i + 1) * B],
            "gamma128": g128,
            "ident": ident,
        }
        for i in range(NCORES)
    ]
    res = run_bass_kernel_spmd(nc, in_maps, list(range(NCORES))).results
    y = np.concatenate([res[i]["y"] for i in range(NCORES)], axis=0)
    return y.reshape(B_TOTAL, C, H, W).astype(np.float32)



# revision 53
# speedup vs baseline: 1.3496x; 1.3496x over previous
"""Trainium2 Bass kernel for the CAM (channel attention) module.

Computes, per batch element b:
    q = x[b].reshape(C, N)                      # C=512, N=4096
    E = q @ q.T                                 # C x C  (symmetric)
    att = softmax(rowmax(E) - E, axis=-1)       # == exp(rowmin - E) / rowsum
    out = gamma * (att @ q) + x[b]

Sharding: data-parallel over batch. 16 batch elements -> 2 per NeuronCore
across 8 cores. gamma replicated. No collectives.

Per-core pipeline (2 batches, software-pipelined):
  - Both batches' input DMAs are issued up front (qnat is double-buffered)
    in 16 column-block chunks per batch, so the DMA engine streams input
    continuously; compute chases the load per 128-column slice j.
  - Per j: 4 PE transposes (f32r data against a bf16 identity -> 1 cyc/row)
    into one PSUM bank, one ACT drain into a rolling 8-slot qt window, then
    4 f32r energy matmuls accumulate E (symmetry: only cols [elo[t]:512],
    missing blocks mirrored by transposing stored ones).
  - Softmax: per-tile row-min (DVE), negate + PE-transpose to a [1,512]
    bf16 row, PE broadcast-matmul ADDS (-min_c) into the E banks, then a
    single ACT exp with scale=-1 per tile produces U = exp(min - E) direct
    from PSUM into fp8e4.  Column scale errors from the bf16 min cancel in
    the normalization R_c = sum_d U[d,c] (PE ones-reduction, fp8).
  - Value matmul att @ q runs in fp8e4 DoubleRow perf mode (0.5 cyc/row,
    two 128-row k-tiles per pass) against an fp8 copy of q (cast on DVE).
    gamma/R_c and the +x residual fuse into one DVE scalar_tensor_tensor
    per output chunk; x enters only here, in exact fp32, so for gamma == 0
    the kernel output is bit-exact x.
  - Batch 0's value/output work is interleaved into batch 1's chase loop so
    PE/DVE/ACT queues stay dependency-sorted and the output DMA stream
    follows the input stream back-to-back.
"""

import sys

import numpy as np

_REPO = "/opt/trn_rl_repo"
if _REPO not in sys.path:
    sys.path.insert(0, _REPO)

B_TOTAL, C, H, W = 16, 512, 64, 64
N = H * W          # 4096
NCORES = 8
B = B_TOTAL // NCORES  # batches per core = 2
CT = C // 128      # 4 c-tiles
NT = N // 128      # 32 n-tiles (j)
NCH = N // 512     # 8 output column chunks per c-tile
INCH = 16          # input chunks per batch (256 cols each)
COLS_PER_CHUNK = N // INCH  # 256

_cache = {}

# Debug knobs (CoreSim error isolation only; both False for the real kernel)
DBG_EXACT_Q = False   # value matmul uses exact f32r q instead of fp8 q8
DBG_EXACT_U = False   # value matmul uses exact f32r U instead of fp8 U8


def _build_program():
    import concourse.bass as bass
    import concourse.bacc as bacc
    import concourse.mybir as mybir
    import concourse.tile as tile
    from contextlib import ExitStack

    f32 = mybir.dt.float32
    f32r = mybir.dt.float32r
    bf16 = mybir.dt.bfloat16
    fp8 = mybir.dt.float8e4
    AX = mybir.AxisListType
    OP = mybir.AluOpType
    ACT = mybir.ActivationFunctionType
    DR = mybir.MatmulPerfMode.DoubleRow

    nc = bacc.Bacc("TRN2", target_bir_lowering=False, debug=False)

    x = nc.dram_tensor("x", [B, C, N], f32r, kind="ExternalInput").ap()
    g128 = nc.dram_tensor("gamma128", [128, 1], f32, kind="ExternalInput").ap()
    ident_d = nc.dram_tensor("ident", [128, 128], f32, kind="ExternalInput").ap()
    y = nc.dram_tensor("y", [B, C, N], f32, kind="ExternalOutput").ap()

    elo = [0, 128, 256, 256]
    mirrors = [(0, 1), (0, 2), (0, 3), (1, 2), (1, 3)]

    with ExitStack() as ctx:
        tc = ctx.enter_context(tile.TileContext(nc))
        const_p = ctx.enter_context(tc.tile_pool(name="const", bufs=1))
        sb = ctx.enter_context(tc.tile_pool(name="sb", bufs=1))
        ps = ctx.enter_context(tc.tile_pool(name="ps", bufs=1, space="PSUM"))

        # ---- constants
        identf = const_p.tile([128, 128], f32, tag="identf")
        nc.sync.dma_start(identf[:], ident_d)
        gam = const_p.tile([128, 1], f32, tag="gam")
        nc.sync.dma_start(gam[:], g128)
        identr = const_p.tile([128, 128], f32r, tag="identr")
        nc.sync.dma_start(identr[:], ident_d.bitcast(f32r))
        ones1 = const_p.tile([1, 128], bf16, tag="ones1")
        nc.gpsimd.memset(ones1[:], 1.0)
        ones2 = const_p.tile([128, 2, 1], fp8, tag="ones2")
        nc.gpsimd.memset(ones2[:], 1.0)
        # prime the ACT function table (Exp+Copy set) off the critical path
        actwarm = const_p.tile([128, 1], f32, tag="actwarm")
        nc.scalar.activation(actwarm[:], gam[:], ACT.Exp)

        # ---- PE warm-up: ramp the clock during the initial DMA wait
        for w in range(5):
            warm = ps.tile([128, 512], f32, tag="TP", bufs=2, name="warm")
            nc.tensor.matmul(
                warm[:, 0:128], identf[:], identf[:],
                start=True, stop=True, skip_group_check=True,
            )

        # ---- input loads: both batches up front (qnat double-buffered)
        qnat = []
        for b in range(B):
            qn = sb.tile([128, CT, N], f32r, tag="qnat", bufs=2, name=f"qnat{b}")
            qnat.append(qn)
            xb = x[b].rearrange("(t p) n -> p t n", t=CT)
            for cch in range(INCH):
                lo = COLS_PER_CHUNK * cch
                hi = lo + COLS_PER_CHUNK
                nc.sync.dma_start(qn[:, :, lo:hi], xb[:, :, lo:hi])

        q8 = [
            sb.tile([128, CT, N], fp8, tag="q8", bufs=2, name=f"q8_{b}")
            for b in range(B)
        ]
        # fp8 residual q - q8 for the value-matmul correction pass.  ONE
        # shared tile: batch 1's writes are ordered after batch 0's value
        # reads of the same byte ranges (n-major value order + emission
        # order below), tracked by subtile deps.
        dq8 = sb.tile([128, CT, N], fp8, tag="dq8", bufs=1, name="dq8")

        def emit_dq_cast(b, cch):
            lo = COLS_PER_CHUNK * cch
            hi = lo + COLS_PER_CHUNK
            nc.gpsimd.tensor_tensor(
                dq8[:, :, lo:hi],
                qnat[b][:, :, lo:hi],
                q8[b][:, :, lo:hi],
                op=OP.subtract,
            )

        def emit_chase(b, interleave):
            """Transpose + energy chase for batch b, software-pipelined one j
            ahead so PE never waits on the ACT drain latency.  `interleave`
            is a list of (min_j, closure) items (prev batch's value work +
            next batch's dq8 casts) popped once their min_j gate passes."""
            E = [
                ps.tile([128, C], f32, tag="E", bufs=4, name=f"E{b}_{t}")
                for t in range(CT)
            ]
            qtjs = [None] * NT

            def emit_tp_drain(j):
                tp = ps.tile([128, C], f32r, tag="TP", bufs=2, name=f"tp{b}_{j}")
                for t in range(CT):
                    nc.tensor.matmul(
                        tp[:, 128 * t:128 * (t + 1)],
                        qnat[b][:, t, 128 * j:128 * (j + 1)],
                        identr[:],
                        is_transpose=True,
                        skip_group_check=True,
                    )
                qtj = sb.tile([128, C], f32r, tag="qtj", bufs=5, name=f"qt{b}_{j}")
                nc.scalar.copy(qtj[:], tp[:])
                qtjs[j] = qtj

            emit_tp_drain(0)
            emit_tp_drain(1)
            for j in range(NT):
                if j + 2 < NT:
                    emit_tp_drain(j + 2)
                qtj = qtjs[j]
                for t in range(CT):
                    nc.tensor.matmul(
                        E[t][:, elo[t]:C],
                        qtj[:, 128 * t:128 * (t + 1)],
                        qtj[:, elo[t]:C],
                        start=(j == 0),
                        stop=(j == NT - 1),
                    )
                if j % 2 == 1:
                    cch = j // 2
                    lo = COLS_PER_CHUNK * cch
                    hi = lo + COLS_PER_CHUNK
                    # fp8 cast of q: batch 0's DVE is idle during its chase;
                    # batch 1's chase also carries batch 0's STT stream, so
                    # alternate its casts between ACT and DVE.
                    eng = nc.scalar if (b == 1 and cch % 2 == 0) else nc.vector
                    if eng is nc.scalar:
                        nc.scalar.copy(q8[b][:, :, lo:hi], qnat[b][:, :, lo:hi])
                    else:
                        nc.vector.tensor_copy(
                            q8[b][:, :, lo:hi], qnat[b][:, :, lo:hi]
                        )
                    if b == 0:
                        # batch 0's dq8 writes are first use of the shared
                        # tile; batch 1's are interleaved after batch 0's
                        # value reads instead.
                        emit_dq_cast(b, cch)
                popped = 0
                while interleave and interleave[0][0] <= j and popped < 3:
                    interleave.pop(0)[1]()
                    popped += 1
            return E

        def emit_softmax(b, E):
            """Mirrors, row-min stats, broadcast-add of -min, exp -> U8.
            Pipelined at 128-column granularity: each rmins column is
            transposed / negated / broadcast into all four E banks as soon as
            it exists, and exp runs per half-tile, so the first value chunks
            (m=0,1) only wait on the first two rmins columns."""
            def emit_mirrors(tt):
                # blk copy (ACT) immediately followed by its PE transpose:
                # pairwise emission keeps Tile's batched semaphore thresholds
                # tight (a mirror only waits for its own blk, not all five)
                for s, t2 in mirrors:
                    if t2 != tt:
                        continue
                    blk = sb.tile([128, 128], f32, tag="blk", bufs=5, name="blk")
                    nc.scalar.copy(blk[:], E[s][:, 128 * tt:128 * (tt + 1)])
                    # start=False: the receiving tile's j=0 start already
                    # marked this (never-written) block pending-zero, so this
                    # lands as a fresh write and leaves the bank uniformly
                    # non-pending for the broadcast-add below.
                    nc.tensor.matmul(
                        E[tt][:, 128 * s:128 * (s + 1)],
                        blk[:],
                        identf[:],
                        is_transpose=True,
                        start=False,
                        stop=True,
                        skip_group_check=True,
                    )

            rmins = sb.tile([128, CT], f32r, tag="rmins", bufs=1, name="rmins")
            tpm = ps.tile([1, C], f32r, tag="O", bufs=2, name="tpm")
            minrow = sb.tile([1, C], bf16, tag="minrow", bufs=1, name="minrow")

            def seg_stats(t):
                """rmins column t -> transpose -> negate to bf16 row seg t."""
                nc.vector.tensor_reduce(
                    rmins[:, t:t + 1], E[t][:], axis=AX.X, op=OP.min
                )
                nc.tensor.matmul(
                    tpm[0:1, 128 * t:128 * (t + 1)],
                    rmins[:, t:t + 1],
                    identr[:],
                    is_transpose=True,
                    skip_group_check=True,
                )
                nc.vector.tensor_scalar_mul(
                    minrow[0:1, 128 * t:128 * (t + 1)],
                    tpm[0:1, 128 * t:128 * (t + 1)],
                    -1.0,
                )

            def bcast(tt, s):
                """E[tt][:, seg s] += broadcast(-min over seg s).  Must come
                after rmins(tt) (WAR on the bank) and minrow seg s."""
                nc.tensor.matmul(
                    E[tt][:, 128 * s:128 * (s + 1)],
                    ones1[:],
                    minrow[0:1, 128 * s:128 * (s + 1)],
                    start=False,
                    stop=True,
                    skip_group_check=True,
                )

            U8 = sb.tile(
                [128, CT, C], f32r if DBG_EXACT_U else fp8,
                tag="U8", bufs=1, name=f"U8_{b}",
            )

            def emit_exp(t, half):
                lo, hi = 256 * half, 256 * (half + 1)
                nc.scalar.activation(
                    U8[:, t, lo:hi], E[t][:, lo:hi], ACT.Exp, scale=-1.0
                )

            # rmins chain first (seg_stats(0) before any post-chase ACT work
            # so Tile's batched semaphore waits stay tight), mirrors pairwise
            # with their blk copies, then all broadcasts, then the exps.
            seg_stats(0)
            emit_mirrors(1)
            seg_stats(1)
            emit_mirrors(2)
            seg_stats(2)
            emit_mirrors(3)
            seg_stats(3)
            # s-major: each wave of four broadcasts becomes ready together
            # when minrow segment s lands
            for s in range(CT):
                for tt in range(CT):
                    bcast(tt, s)
            for t in range(CT):
                emit_exp(t, 0)
            # h1 exps t-major: releases E bank t for the next batch's energy
            # accumulation as early as possible
            for t in range(CT):
                emit_exp(t, 1)
            return U8

        def emit_value_work(b, U8, otag="O", obufs=2):
            """Returns a list of closures: R/scale chain + 32 output chunks."""
            work = []
            sc = [None] * CT

            Rall_box = [None]

            def do_R(ms):
                def fn():
                    if Rall_box[0] is None:
                        Rall_box[0] = ps.tile(
                            [128, CT], f32, tag="O", bufs=2, name=f"R{b}"
                        )
                    Rall = Rall_box[0]
                    for m in ms:
                        if DBG_EXACT_U:
                            for k in range(CT):
                                nc.tensor.matmul(
                                    Rall[:, m:m + 1],
                                    U8[:, k, 128 * m:128 * (m + 1)],
                                    ones2[:, 0, :],
                                    start=(k == 0),
                                    stop=(k == CT - 1),
                                    skip_group_check=True,
                                )
                        else:
                            for p in range(2):
                                nc.tensor.matmul(
                                    Rall[:, m:m + 1],
                                    U8[:, 2 * p:2 * p + 2, 128 * m:128 * (m + 1)],
                                    ones2[:],
                                    perf_mode=DR,
                                    start=(p == 0),
                                    stop=(p == 1),
                                    skip_group_check=True,
                                )
                    for m in ms:
                        Rsb = sb.tile([128, 1], f32, tag="rsb", bufs=2, name="Rsb")
                        nc.vector.tensor_scalar_max(
                            Rsb[:], Rall[:, m:m + 1], 1e-38
                        )
                        rec = sb.tile([128, 1], f32, tag="rec", bufs=2, name="rec")
                        nc.vector.reciprocal(rec[:], Rsb[:])
                        scm = sb.tile([128, 1], f32, tag="sc", bufs=8, name=f"sc{m}")
                        nc.vector.tensor_scalar_mul(scm[:], rec[:], gam[:, 0:1])
                        sc[m] = scm
                return fn

            work.append((6, do_R((0, 1))))
            work.append((6, do_R((2, 3))))

            def chunk(m, n):
                def do_chunk():
                    O = ps.tile([128, 512], f32, tag=otag, bufs=obufs, name=f"O{b}")
                    if DBG_EXACT_Q or DBG_EXACT_U:
                        for k in range(CT):
                            rhs = (
                                qnat[b][:, k, 512 * n:512 * (n + 1)]
                                if DBG_EXACT_Q
                                else q8[b][:, k, 512 * n:512 * (n + 1)]
                            )
                            nc.tensor.matmul(
                                O[:],
                                U8[:, k, 128 * m:128 * (m + 1)],
                                rhs,
                                start=(k == 0),
                                stop=(k == CT - 1),
                                skip_group_check=True,
                            )
                    else:
                        # out = U8^T (q8 + dq8): main fp8 pass + fp8 residual
                        # correction pass, one PSUM accumulation group.
                        for gi, src in enumerate((q8[b], dq8)):
                            for p in range(2):
                                nc.tensor.matmul(
                                    O[:],
                                    U8[:, 2 * p:2 * p + 2, 128 * m:128 * (m + 1)],
                                    src[:, 2 * p:2 * p + 2, 512 * n:512 * (n + 1)],
                                    perf_mode=DR,
                                    start=(gi == 0 and p == 0),
                                    stop=(gi == 1 and p == 1),
                                    skip_group_check=True,
                                )
                    osb = sb.tile([128, 512], f32, tag="osb", bufs=5, name="osb")
                    nc.vector.scalar_tensor_tensor(
                        osb[:],
                        O[:],
                        sc[m][:],
                        qnat[b][:, m, 512 * n:512 * (n + 1)],
                        op0=OP.mult,
                        op1=OP.add,
                    )
                    nc.sync.dma_start(
                        y[b, 128 * m:128 * (m + 1), 512 * n:512 * (n + 1)],
                        osb[:],
                    )
                return do_chunk

            # n-major: the shared dq8's columns are released in n order so
            # the NEXT batch's dq8 casts (appended after each n group when
            # `next_b` is set) can chase.  min_j gates pace the pops so the
            # next batch's chase (which owns PE) isn't flooded early: output
            # chunk k only needs to be ready when the out-DMA stream (which
            # starts after the input stream finishes) reaches it.
            next_b = b + 1 if b + 1 < B else None
            ki = 0
            for_spill = NT + 1  # never pops in-loop; emitted after softmax
            for n in range(NCH):
                for m in range(CT):
                    # last 3 chunks spill past the chase so the final STT
                    # convoy doesn't sit ahead of the next batch's softmax
                    # reductions in the DVE queue
                    gate = 8 + (ki * 23) // 32 if ki < 29 else for_spill
                    work.append((gate, chunk(m, n)))
                    ki += 1
                if next_b is not None:
                    gate = 9 + (ki * 23) // 32
                    for cch in (2 * n, 2 * n + 1):
                        # also never before the next batch's q8 cast for this
                        # chunk (emitted at its chase j == 2*cch + 1)
                        work.append(
                            (max(gate, 2 * cch + 2),
                             lambda bb=next_b, cc=cch: emit_dq_cast(bb, cc))
                        )
            return work

        # ---- batch 0: chase, softmax; value work interleaved into batch 1
        E0 = emit_chase(0, None)
        U80 = emit_softmax(0, E0)
        pending = emit_value_work(0, U80)
        E1 = emit_chase(1, pending)
        # batch 1's softmax stats go ahead of any spilled batch-0 output
        # chunks so its DVE reductions aren't stuck behind their STTs
        U81 = emit_softmax(1, E1)
        for _, w_ in pending:
            w_()
        # last batch's value chunks rotate through the (now dead) E banks:
        # 4-deep PSUM buffering instead of 2 shortens the output stream
        for _, w_ in emit_value_work(1, U81, otag="E", obufs=4):
            w_()

    nc.compile()
    return nc


def get_program():
    if "nc" not in _cache:
        _cache["nc"] = _build_program()
    return _cache["nc"]


def kernel(x, gamma):
    from concourse.bass_utils import run_bass_kernel_spmd

    nc = get_program()
    xr = np.ascontiguousarray(
        np.asarray(x, dtype=np.float32).reshape(B_TOTAL, C, N)
    )
    g = np.asarray(gamma, dtype=np.float32).reshape(1)
    g128 = np.ascontiguousarray(
        np.broadcast_to(g.reshape(1, 1), (128, 1))
    ).astype(np.float32)
    ident = np.eye(128, dtype=np.float32)
    in_maps = [
        {
            "x": xr[i * B:(i + 1) * B],
            "gamma128": g128,
            "ident": ident,
        }
        for i in range(NCORES)
    ]
    res = run_bass_kernel_spmd(nc, in_maps, list(range(NCORES))).results
    yout = np.concatenate([res[i]["y"] for i in range(NCORES)], axis=0)
    return yout.reshape(B_TOTAL, C, H, W).astype(np.float32)
